# revision 3
# baseline (speedup 1.0000x reference)
"""Trainium2 Bass kernel for nn_MoEClassifier (dense top-2 MoE classifier).

Strategy: data-parallel across 8 NeuronCores — each core runs the full model
on 1/8 of the batch (1024 rows). All activations are kept feature-major
([features, batch]) so every layer is matmul(lhsT=W[in,out], rhs=actT) with
weights stationary. No collectives are needed.

Routing (feature extractor, kw projection, cosine top-2) runs in fp32 so the
expert selection matches the fp32 reference; the expert MLPs run in
float32r (1 cycle/row on the PE at N>=256, ~1.3e-4 rel err).

Self-contained: hardcodes all shapes; host-side prep only reshapes/slices.
"""
import sys
import time

sys.path.insert(0, "/opt/trn_rl_repo")

import numpy as np

import concourse.bacc as bacc
import concourse.tile as tile
from concourse import mybir
from concourse.bass_utils import run_bass_kernel_spmd

AF = mybir.ActivationFunctionType
ALU = mybir.AluOpType
F32 = mybir.dt.float32
F32R = mybir.dt.float32r
BF16 = mybir.dt.bfloat16
I32 = mybir.dt.int32
U32 = mybir.dt.uint32

P = 128
NCORES = 8
B, D, H, C, K, E = 8192, 1024, 2048, 1000, 128, 8
BC = B // NCORES          # batch rows per core (1024)
HH = H // 2               # 1024
CP = 1024                 # C padded to 1024
NH = 512                  # batch half processed per pass
DKT, HKT, HHKT = D // P, H // P, HH // P   # 8, 16, 8 k-tiles
HMT, HHMT, CMT = H // P, HH // P, CP // P  # 16, 8, 8 m-tiles

# dtype of the expert-MLP matmuls: "f32", "f32r" or "bf16".
DT_EXPERT = "f32r"
_DT = {"f32": F32, "f32r": F32R, "bf16": BF16}


def build_nc():
    dt_e = _DT[DT_EXPERT]
    nc = bacc.Bacc("TRN2", target_bir_lowering=False)

    xT = nc.dram_tensor("xT", [D, BC], F32, kind="ExternalInput")
    fw1t = nc.dram_tensor("fw1t", [HMT, P, DKT, P], F32, kind="ExternalInput")
    fb1 = nc.dram_tensor("fb1", [H], F32, kind="ExternalInput")
    fw2t = nc.dram_tensor("fw2t", [HMT, P, HKT, P], F32, kind="ExternalInput")
    fb2 = nc.dram_tensor("fb2", [H], F32, kind="ExternalInput")
    kwt = nc.dram_tensor("kwt", [1, P, HKT, P], F32, kind="ExternalInput")
    kb = nc.dram_tensor("kb", [K], F32, kind="ExternalInput")
    keys = nc.dram_tensor("keys", [E, K], F32, kind="ExternalInput")
    ew1t = nc.dram_tensor("ew1t", [E, HMT, P, HKT, P], dt_e, kind="ExternalInput")
    eb1 = nc.dram_tensor("eb1", [E, H], F32, kind="ExternalInput")
    ew2t = nc.dram_tensor("ew2t", [E, HHMT, P, HKT, P], dt_e, kind="ExternalInput")
    eb2 = nc.dram_tensor("eb2", [E, HH], F32, kind="ExternalInput")
    ew3t = nc.dram_tensor("ew3t", [E, CMT, P, HHKT, P], dt_e, kind="ExternalInput")
    eb3p = nc.dram_tensor("eb3p", [E, CP], F32, kind="ExternalInput")
    iota8 = nc.dram_tensor("iota8", [E], F32, kind="ExternalInput")
    ident = nc.dram_tensor("ident", [P, P], F32, kind="ExternalInput")

    finalT = nc.dram_tensor("finalT", [CP, BC], F32, kind="ExternalOutput")
    wout = nc.dram_tensor("wout", [BC, E], F32, kind="ExternalOutput")
    iout = nc.dram_tensor("iout", [BC, 2], I32, kind="ExternalOutput")
    simout = nc.dram_tensor("simout", [BC, E], F32, kind="ExternalOutput")

    with tile.TileContext(nc) as tc:
        _build(nc, tc, dt_e, locals())
    nc.compile()
    return nc


def _build(nc, tc, dt_e, t):
    xT, fw1t, fb1, fw2t, fb2, kwt, kb, keys = (
        t["xT"], t["fw1t"], t["fb1"], t["fw2t"], t["fb2"], t["kwt"], t["kb"],
        t["keys"])
    ew1t, eb1, ew2t, eb2, ew3t, eb3p = (
        t["ew1t"], t["eb1"], t["ew2t"], t["eb2"], t["ew3t"], t["eb3p"])
    iota8, ident = t["iota8"], t["ident"]
    finalT, wout, iout, simout = t["finalT"], t["wout"], t["iout"], t["simout"]

    from contextlib import ExitStack
    ctx = ExitStack()
    with ctx:
        p_const = ctx.enter_context(tc.tile_pool(name="p_const", bufs=1))
        p_bias = ctx.enter_context(tc.tile_pool(name="p_bias", bufs=2))
        p_w = ctx.enter_context(tc.tile_pool(name="p_w", bufs=3))
        p_feats = ctx.enter_context(tc.tile_pool(name="p_feats", bufs=1))
        p_ps = ctx.enter_context(tc.tile_pool(name="p_ps", bufs=4, space="PSUM"))
        p_psr = ctx.enter_context(tc.tile_pool(name="p_psr", bufs=2, space="PSUM"))

        feats = p_feats.tile([P, HKT, BC], F32)

        # constants (persist the whole kernel)
        ident_sb = p_const.tile([P, P], F32)
        nc.sync.dma_start(ident_sb[:], ident[:])
        ones1 = p_const.tile([1, P], F32)
        nc.vector.memset(ones1[:], 1.0)
        ones128 = p_const.tile([P, 1], F32)
        nc.vector.memset(ones128[:], 1.0)
        iota_sb = p_const.tile([1, E], F32)
        nc.sync.dma_start(iota_sb[:], iota8.rearrange("(a e) -> a e", a=1))
        eb3_sb = p_const.tile([E, CP], F32)
        nc.sync.dma_start(eb3_sb[:], eb3p[:])
        weightsT = p_const.tile([E, BC], F32)

        def load_bias(bias_ap, mts):
            b = p_bias.tile([P, mts], F32, tag="bias")
            nc.sync.dma_start(b[:], bias_ap.rearrange("(mt p) -> p mt", p=P))
            return b

        def mm_chain(ps, wcol, kts, rhs_fn):
            for k in range(kts):
                nc.tensor.matmul(ps[:], wcol[:, k], rhs_fn(k),
                                 start=(k == 0), stop=(k == kts - 1))

        # ---- feature extractor (per batch half to bound SBUF) ----
        with tc.tile_pool(name="p_l12", bufs=1) as p_l12:
            x_sb = p_l12.tile([P, DKT, BC], F32, tag="x")
            nc.sync.dma_start(x_sb[:], xT.rearrange("(k p) b -> p k b", p=P))
            for nh2 in range(2):
                ns = slice(nh2 * NH, (nh2 + 1) * NH)
                r1h = p_l12.tile([P, HMT, NH], F32, tag="r1h")
                b1 = load_bias(fb1, HMT)
                for mt in range(HMT):
                    wcol = p_w.tile([P, DKT, P], F32, tag="wcol")
                    nc.sync.dma_start(wcol[:], fw1t[mt])
                    ps = p_ps.tile([P, NH], F32, tag="mmps")
                    mm_chain(ps, wcol, DKT, lambda k: x_sb[:, k, ns])
                    nc.scalar.activation(r1h[:, mt], ps[:], AF.Relu,
                                         bias=b1[:, mt:mt + 1])
                b2 = load_bias(fb2, HMT)
                for mt in range(HMT):
                    wcol = p_w.tile([P, HKT, P], F32, tag="wcol")
                    nc.sync.dma_start(wcol[:], fw2t[mt])
                    ps = p_ps.tile([P, NH], F32, tag="mmps")
                    mm_chain(ps, wcol, HKT, lambda k: r1h[:, k])
                    nc.scalar.activation(feats[:, mt, ns], ps[:], AF.Identity,
                                         bias=b2[:, mt:mt + 1])

        # ---- routing ----
        with tc.tile_pool(name="p_route", bufs=2) as p_route:
            pk = p_route.tile([P, BC], F32, tag="pk")
            bk = load_bias(kb, 1)
            kwcol = p_w.tile([P, HKT, P], F32, tag="wcol")
            nc.sync.dma_start(kwcol[:], kwt[0])
            for n in range(2):
                ps = p_ps.tile([P, NH], F32, tag="mmps")
                mm_chain(ps, kwcol, HKT,
                         lambda k: feats[:, k, n * NH:(n + 1) * NH])
                nc.scalar.activation(pk[:, n * NH:(n + 1) * NH], ps[:],
                                     AF.Identity, bias=bk[:, 0:1])

            pksq = p_route.tile([P, BC], F32, tag="pksq")
            nc.vector.tensor_tensor(pksq[:], pk[:], pk[:], ALU.mult)

            keys_sb = p_route.tile([E, K], F32, tag="keys")
            nc.sync.dma_start(keys_sb[:], keys[:])
            ksq = p_route.tile([E, K], F32, tag="ksq")
            nc.vector.tensor_tensor(ksq[:], keys_sb[:], keys_sb[:], ALU.mult)
            kss = p_route.tile([E, 1], F32, tag="kss")
            nc.vector.reduce_sum(kss[:], ksq[:], axis=mybir.AxisListType.X)
            ksr = p_route.tile([E, 1], F32, tag="ksr")
            nc.scalar.activation(ksr[:], kss[:], AF.Sqrt)
            kin = p_route.tile([E, 1], F32, tag="kin")
            nc.vector.reciprocal(kin[:], ksr[:])
            keysn = p_route.tile([E, K], F32, tag="keysn")
            nc.vector.tensor_scalar_mul(keysn[:], keys_sb[:], kin[:])
            keysnT_ps = p_psr.tile([P, E], F32, tag="rps")
            nc.tensor.transpose(keysnT_ps[:], keysn[:], ident_sb[:E, :E])
            keysnT = p_route.tile([P, E], F32, tag="keysnT")
            nc.scalar.copy(keysnT[:], keysnT_ps[:])

            iota_ps = p_psr.tile([P, E], F32, tag="rps")
            nc.tensor.matmul(iota_ps[:], ones1[:], iota_sb[:], start=True,
                             stop=True)
            iota_bc = p_route.tile([P, E], F32, tag="iota_bc")
            nc.scalar.copy(iota_bc[:], iota_ps[:])

            for bt in range(BC // P):
                bs = slice(bt * P, (bt + 1) * P)
                ssq_ps = p_psr.tile([P, 1], F32, tag="rps")
                nc.tensor.matmul(ssq_ps[:], pksq[:, bs], ones128[:],
                                 start=True, stop=True)
                nrm = p_route.tile([P, 1], F32, tag="nrm")
                nc.scalar.activation(nrm[:], ssq_ps[:], AF.Sqrt)
                inv = p_route.tile([P, 1], F32, tag="inv")
                nc.vector.reciprocal(inv[:], nrm[:])

                sim_ps = p_psr.tile([P, E], F32, tag="rps")
                nc.tensor.matmul(sim_ps[:], pk[:, bs], keysnT[:], start=True,
                                 stop=True)
                sim_sb = p_route.tile([P, E], F32, tag="sim_sb")
                nc.vector.tensor_scalar_mul(sim_sb[:], sim_ps[:], inv[:])
                nc.sync.dma_start(simout[bs, :], sim_sb[:])

                vmax = p_route.tile([P, E], F32, tag="vmax")
                nc.vector.max(vmax[:], sim_sb[:])
                vidx = p_route.tile([P, E], U32, tag="vidx")
                nc.vector.max_index(vidx[:], vmax[:], sim_sb[:])

                negv1 = p_route.tile([P, 1], F32, tag="negv1")
                nc.vector.tensor_scalar_mul(negv1[:], vmax[:, 0:1], -1.0)
                e2 = p_route.tile([P, 1], F32, tag="e2")
                nc.scalar.activation(e2[:], vmax[:, 1:2], AF.Exp, bias=negv1[:])
                den = p_route.tile([P, 1], F32, tag="den")
                nc.vector.tensor_scalar_add(den[:], e2[:], 1.0)
                invd = p_route.tile([P, 1], F32, tag="invd")
                nc.vector.reciprocal(invd[:], den[:])
                sm2 = p_route.tile([P, 1], F32, tag="sm2")
                nc.vector.tensor_tensor(sm2[:], e2[:], invd[:], ALU.mult)

                idxf = p_route.tile([P, 2], F32, tag="idxf")
                nc.vector.tensor_copy(idxf[:], vidx[:, 0:2])
                iout_t = p_route.tile([P, 2], I32, tag="iout_t")
                nc.vector.tensor_copy(iout_t[:], vidx[:, 0:2])
                nc.sync.dma_start(iout[bs, :], iout_t[:])

                w1 = p_route.tile([P, E], F32, tag="w1")
                nc.vector.tensor_scalar(w1[:], iota_bc[:], idxf[:, 0:1],
                                        invd[:], ALU.is_equal, ALU.mult)
                w2 = p_route.tile([P, E], F32, tag="w2")
                nc.vector.tensor_scalar(w2[:], iota_bc[:], idxf[:, 1:2],
                                        sm2[:], ALU.is_equal, ALU.mult)
                w_sb = p_route.tile([P, E], F32, tag="w_sb")
                nc.vector.tensor_tensor(w_sb[:], w1[:], w2[:], ALU.add)
                nc.sync.dma_start(wout[bs, :], w_sb[:])

                wT_ps = p_psr.tile([E, P], F32, tag="rps")
                nc.tensor.transpose(wT_ps[:], w_sb[:], ident_sb[:])
                nc.scalar.copy(weightsT[:, bs], wT_ps[:])

        # ---- experts ----
        with tc.tile_pool(name="p_fr", bufs=1) as p_fr, \
             tc.tile_pool(name="p_h1", bufs=1) as p_h1, \
             tc.tile_pool(name="p_h2w", bufs=1) as p_h2w, \
             tc.tile_pool(name="p_fin", bufs=1) as p_fin, \
             tc.tile_pool(name="p_wb", bufs=2) as p_wb:
            for nh in range(2):
                ns = slice(nh * NH, (nh + 1) * NH)
                if dt_e != F32:
                    featsr = p_fr.tile([P, HKT, NH], dt_e, tag="featsr")
                    nc.scalar.activation(featsr[:], feats[:, :, ns],
                                         AF.Identity)
                else:
                    featsr = None

                fin = p_fin.tile([P, CMT, NH], F32, tag="fin")
                nc.vector.memset(fin[:], 0.0)

                for e in range(E):
                    eb1_sb = p_bias.tile([P, HMT], F32, tag="bias")
                    nc.sync.dma_start(
                        eb1_sb[:], eb1[e].rearrange("(mt p) -> p mt", p=P))
                    eb2_sb = p_bias.tile([P, HHMT], F32, tag="bias")
                    nc.sync.dma_start(
                        eb2_sb[:], eb2[e].rearrange("(mt p) -> p mt", p=P))

                    # w broadcast [P, NH] for this expert / batch half
                    # (stage row e at partition 0 — PE needs base partition 0)
                    wrow = p_wb.tile([1, NH], F32, tag="wrow")
                    nc.sync.dma_start(wrow[:], weightsT[e:e + 1, ns])
                    wb_ps = p_ps.tile([P, NH], F32, tag="mmps")
                    nc.tensor.matmul(wb_ps[:], ones1[:], wrow[:], start=True,
                                     stop=True)
                    wb = p_wb.tile([P, NH], dt_e, tag="wb")
                    nc.scalar.copy(wb[:], wb_ps[:])

                    h1 = p_h1.tile([P, HMT, NH], dt_e, tag="h1")
                    for mt in range(HMT):
                        wcol = p_w.tile([P, HKT, P], dt_e, tag="wcol")
                        nc.sync.dma_start(wcol[:], ew1t[e, mt])
                        ps = p_ps.tile([P, NH], F32, tag="mmps")
                        if featsr is None:
                            mm_chain(ps, wcol, HKT, lambda k: feats[:, k, ns])
                        else:
                            mm_chain(ps, wcol, HKT, lambda k: featsr[:, k])
                        nc.scalar.activation(h1[:, mt], ps[:], AF.Relu,
                                             bias=eb1_sb[:, mt:mt + 1])

                    h2w = p_h2w.tile([P, HHMT, NH], dt_e, tag="h2w")
                    for mt in range(HHMT):
                        wcol = p_w.tile([P, HKT, P], dt_e, tag="wcol")
                        nc.sync.dma_start(wcol[:], ew2t[e, mt])
                        ps = p_ps.tile([P, NH], F32, tag="mmps")
                        mm_chain(ps, wcol, HKT, lambda k: h1[:, k])
                        # relu(ps + eb2) then * w
                        nc.vector.tensor_scalar(h2w[:, mt], ps[:],
                                                eb2_sb[:, mt:mt + 1], 0.0,
                                                ALU.add, ALU.max)
                        nc.vector.tensor_tensor(h2w[:, mt], h2w[:, mt], wb[:],
                                                ALU.mult)

                    for mt in range(CMT):
                        wcol = p_w.tile([P, HHKT, P], dt_e, tag="wcol")
                        nc.sync.dma_start(wcol[:], ew3t[e, mt])
                        ps = p_ps.tile([P, NH], F32, tag="mmps")
                        mm_chain(ps, wcol, HHKT, lambda k: h2w[:, k])
                        nc.vector.tensor_tensor(fin[:, mt], fin[:, mt], ps[:],
                                                ALU.add)

                # bias-through-routing term and writeback
                for mt in range(CMT):
                    ps = p_ps.tile([P, NH], F32, tag="mmps")
                    nc.tensor.matmul(ps[:], eb3_sb[:, mt * P:(mt + 1) * P],
                                     weightsT[:, ns], start=True, stop=True)
                    nc.vector.tensor_tensor(fin[:, mt], fin[:, mt], ps[:],
                                            ALU.add)
                    nc.sync.dma_start(finalT[mt * P:(mt + 1) * P, ns],
                                      fin[:, mt])


def _retile(w, dt):
    """[K, M] f32 -> [M/P, P(of K), K/P, P(of M)] in dt."""
    Kd, Md = w.shape
    r = w.reshape(Kd // P, P, Md // P, P).transpose(2, 1, 0, 3)
    return np.ascontiguousarray(r).astype(dt, copy=False)


def _retile_e(w, dt):
    Ed, Kd, Md = w.shape
    r = w.reshape(Ed, Kd // P, P, Md // P, P).transpose(0, 3, 2, 1, 4)
    return np.ascontiguousarray(r).astype(dt, copy=False)


_CACHE = {}


def _get_nc():
    if "nc" not in _CACHE:
        _CACHE["nc"] = build_nc()
    return _CACHE["nc"]


def _prep_shared(inputs):
    i = {k: np.asarray(v, dtype=np.float32) for k, v in inputs.items()}
    ew3 = np.zeros((E, HH, CP), np.float32)
    ew3[:, :, :C] = i["ew3"]
    eb3p = np.zeros((E, CP), np.float32)
    eb3p[:, :C] = i["eb3"]
    if DT_EXPERT == "bf16":
        import ml_dtypes
        edt = ml_dtypes.bfloat16
    else:
        edt = np.float32
    return {
        "fw1t": _retile(i["fw1"], np.float32),
        "fb1": i["fb1"],
        "fw2t": _retile(i["fw2"], np.float32),
        "fb2": i["fb2"],
        "kwt": _retile(i["kw"], np.float32),
        "kb": i["kb"],
        "keys": i["keys"],
        "ew1t": _retile_e(i["ew1"], edt),
        "eb1": i["eb1"],
        "ew2t": _retile_e(i["ew2"], edt),
        "eb2": i["eb2"],
        "ew3t": _retile_e(ew3, edt),
        "eb3p": eb3p,
        "iota8": np.arange(E, dtype=np.float32),
        "ident": np.eye(P, dtype=np.float32),
    }


def _prep_inputs(inputs):
    shared = _prep_shared(inputs)
    xTf = np.ascontiguousarray(np.asarray(inputs["x"], np.float32).T)
    in_maps = []
    for c in range(NCORES):
        m = dict(shared)
        m["xT"] = np.ascontiguousarray(xTf[:, c * BC:(c + 1) * BC])
        in_maps.append(m)
    return in_maps


def _assemble(results):
    final = np.concatenate([r["finalT"][:C, :].T for r in results], axis=0)
    weights = np.concatenate([r["wout"] for r in results], axis=0)
    top_i = np.concatenate([r["iout"] for r in results], axis=0)
    sim = np.concatenate([r["simout"] for r in results], axis=0)
    return final, weights, top_i, sim


# ---------------------------------------------------------------------------
# Staged runner (mirrors concourse.bass2jax.run_bass_via_pjrt, but keeps the
# jitted executable and device-resident inputs so repeat calls only execute).
# ---------------------------------------------------------------------------

def _make_runner():
    import jax
    from jax.experimental.shard_map import shard_map
    from jax.sharding import Mesh, PartitionSpec
    from concourse import bass2jax

    nc = _get_nc()
    bass2jax.install_neuronx_cc_hook()
    pname = nc.partition_id_tensor.name if nc.partition_id_tensor else None
    in_names, out_names, out_avals, zero_outs = [], [], [], []
    for alloc in nc.m.functions[0].allocations:
        if not isinstance(alloc, mybir.MemoryLocationSet):
            continue
        name = alloc.memorylocations[0].name
        if alloc.kind == "ExternalInput":
            if name != pname:
                in_names.append(name)
        elif alloc.kind == "ExternalOutput":
            out_names.append(name)
            shape = tuple(alloc.tensor_shape)
            dtype = mybir.dt.np(alloc.dtype)
            out_avals.append(jax.core.ShapedArray(shape, dtype))
            zero_outs.append(np.zeros(shape, dtype))
    n_params = len(in_names)
    all_names = list(in_names) + list(out_names) + ([pname] if pname else [])

    def _body(*args):
        operands = list(args)
        if pname is not None:
            operands.append(bass2jax.partition_id_tensor())
        outs = bass2jax._bass_exec_p.bind(
            *operands,
            out_avals=tuple(out_avals),
            in_names=tuple(all_names),
            out_names=tuple(out_names),
            lowering_input_output_aliases=(),
            sim_require_finite=True,
            sim_require_nnan=True,
            nc=nc,
        )
        return tuple(outs)

    devices = jax.devices()[:NCORES]
    mesh = Mesh(np.asarray(devices), ("core",))
    in_specs = (PartitionSpec("core"),) * (n_params + len(out_names))
    out_specs = (PartitionSpec("core"),) * len(out_names)
    fn = jax.jit(
        shard_map(_body, mesh=mesh, in_specs=in_specs, out_specs=out_specs,
                  check_rep=False),
        keep_unused=True,
    )
    return dict(fn=fn, mesh=mesh, in_names=in_names, out_names=out_names,
                zero_outs=zero_outs, out_avals=out_avals)


def _get_runner():
    if "runner" not in _CACHE:
        _CACHE["runner"] = _make_runner()
    return _CACHE["runner"]


def _stage(in_maps):
    import jax
    from jax.sharding import NamedSharding, PartitionSpec
    r = _get_runner()
    sh = NamedSharding(r["mesh"], PartitionSpec("core"))
    args = []
    for n in r["in_names"]:
        a = np.concatenate([np.asarray(m[n]) for m in in_maps], axis=0)
        args.append(jax.device_put(a, sh))
    for z in r["zero_outs"]:
        g = np.zeros((NCORES * z.shape[0], *z.shape[1:]), z.dtype)
        args.append(jax.device_put(g, sh))
    return args


def _run(args):
    r = _get_runner()
    outs = r["fn"](*args)
    res = []
    for c in range(NCORES):
        res.append({
            n: np.asarray(outs[i]).reshape(NCORES, *r["out_avals"][i].shape)[c]
            for i, n in enumerate(r["out_names"])
        })
    return outs, res


def kernel(**inputs):
    in_maps = _prep_inputs(inputs)
    args = _stage(in_maps)
    _, res = _run(args)
    return _assemble(res)


def time_kernel(inputs, iters=10):
    """Stages inputs on device, then times repeated executions (seconds)."""
    import jax
    in_maps = _prep_inputs(inputs)
    args = _stage(in_maps)
    r = _get_runner()
    outs = r["fn"](*args)          # warmup (compile happens on first call)
    jax.block_until_ready(outs)
    times = []
    for _ in range(iters):
        t0 = time.perf_counter()
        outs = r["fn"](*args)
        jax.block_until_ready(outs)
        times.append(time.perf_counter() - t0)
    return min(times), sorted(times)[len(times) // 2]


if __name__ == "__main__":
    t0 = time.time()
    nc = _get_nc()
    print("built in", round(time.time() - t0, 1), "s")


# revision 18
# speedup vs baseline: 24.6978x; 24.6978x over previous
"""Trainium2 Bass kernel for nn_MoEClassifier (dense top-2 MoE classifier).

Strategy: data-parallel across 8 NeuronCores — each core runs the full model
on 1/8 of the batch (1024 rows). All activations are kept feature-major
([features, batch]) so every layer is matmul(lhsT=W[in,out], rhs=actT) with
weights stationary. No collectives are needed.

Routing (feature extractor, kw projection, cosine top-2) runs in fp32 so the
expert selection matches the fp32 reference; the expert MLPs run in
float32r (1 cycle/row on the PE at N>=256, ~1.3e-4 rel err).

Self-contained: hardcodes all shapes; host-side prep only reshapes/slices.
"""
import sys
import time

sys.path.insert(0, "/opt/trn_rl_repo")

import numpy as np

import concourse.bacc as bacc
import concourse.tile as tile
from concourse import mybir
from concourse.bass_utils import run_bass_kernel_spmd

AF = mybir.ActivationFunctionType
ALU = mybir.AluOpType
F32 = mybir.dt.float32
F32R = mybir.dt.float32r
BF16 = mybir.dt.bfloat16
I32 = mybir.dt.int32
U32 = mybir.dt.uint32

P = 128
NCORES = 8
B, D, H, C, K, E = 8192, 1024, 2048, 1000, 128, 8
BC = B // NCORES          # batch rows per core (1024)
HH = H // 2               # 1024
CP = 1024                 # C padded to 1024
NH = 512                  # batch half processed per pass
DKT, HKT, HHKT = D // P, H // P, HH // P   # 8, 16, 8 k-tiles
HMT, HHMT, CMT = H // P, HH // P, CP // P  # 16, 8, 8 m-tiles

# dtype of the expert-MLP matmuls: "f32", "f32r" or "bf16".
DT_EXPERT = "f32r"
_DT = {"f32": F32, "f32r": F32R, "bf16": BF16}


def build_nc(reps=1):
    dt_e = _DT[DT_EXPERT]
    nc = bacc.Bacc("TRN2", target_bir_lowering=False)

    xT = nc.dram_tensor("xT", [D, BC], F32, kind="ExternalInput")
    fw1t = nc.dram_tensor("fw1t", [HMT, P, DKT, P], F32, kind="ExternalInput")
    fb1 = nc.dram_tensor("fb1", [H], F32, kind="ExternalInput")
    fw2t = nc.dram_tensor("fw2t", [HMT, P, HKT, P], dt_e, kind="ExternalInput")
    fw2Tt = nc.dram_tensor("fw2Tt", [HMT, P, HKT, P], F32, kind="ExternalInput")
    fb2 = nc.dram_tensor("fb2", [H], F32, kind="ExternalInput")
    kwt = nc.dram_tensor("kwt", [1, P, HKT, P], F32, kind="ExternalInput")
    kb = nc.dram_tensor("kb", [K], F32, kind="ExternalInput")
    keys = nc.dram_tensor("keys", [E, K], F32, kind="ExternalInput")
    ew1t = nc.dram_tensor("ew1t", [E, HMT, P, HKT, P], dt_e, kind="ExternalInput")
    eb1 = nc.dram_tensor("eb1", [E, H], F32, kind="ExternalInput")
    ew2t = nc.dram_tensor("ew2t", [E, HHMT, P, HKT, P], dt_e, kind="ExternalInput")
    eb2 = nc.dram_tensor("eb2", [E, HH], F32, kind="ExternalInput")
    ew3t = nc.dram_tensor("ew3t", [E, CMT, P, HHKT, P], dt_e, kind="ExternalInput")
    eb3p = nc.dram_tensor("eb3p", [E, CP], F32, kind="ExternalInput")
    iota8 = nc.dram_tensor("iota8", [E], F32, kind="ExternalInput")
    ident = nc.dram_tensor("ident", [P, P], F32, kind="ExternalInput")

    finalT = nc.dram_tensor("finalT", [CP, BC], F32, kind="ExternalOutput")
    wout = nc.dram_tensor("wout", [BC, E], F32, kind="ExternalOutput")
    iout = nc.dram_tensor("iout", [BC, 2], I32, kind="ExternalOutput")
    simout = nc.dram_tensor("simout", [BC, E], F32, kind="ExternalOutput")

    tensors = dict(locals())
    with tile.TileContext(nc) as tc:
        for _ in range(reps):
            _build(nc, tc, dt_e, tensors)
    nc.compile()
    return nc


def _build(nc, tc, dt_e, t):
    xT, fw1t, fb1, fw2t, fb2, kwt, kb, keys = (
        t["xT"], t["fw1t"], t["fb1"], t["fw2t"], t["fb2"], t["kwt"], t["kb"],
        t["keys"])
    fw2Tt = t["fw2Tt"]
    ew1t, eb1, ew2t, eb2, ew3t, eb3p = (
        t["ew1t"], t["eb1"], t["ew2t"], t["eb2"], t["ew3t"], t["eb3p"])
    iota8, ident = t["iota8"], t["ident"]
    finalT, wout, iout, simout = t["finalT"], t["wout"], t["iout"], t["simout"]

    from contextlib import ExitStack
    ctx = ExitStack()
    with ctx:
        p_const = ctx.enter_context(tc.tile_pool(name="p_const", bufs=1))
        p_bias = ctx.enter_context(tc.tile_pool(name="p_bias", bufs=2))
        p_w = ctx.enter_context(tc.tile_pool(name="p_w", bufs=3))
        p_feats = ctx.enter_context(tc.tile_pool(name="p_feats", bufs=1))
        p_ps = ctx.enter_context(tc.tile_pool(name="p_ps", bufs=4, space="PSUM"))
        p_psr = ctx.enter_context(tc.tile_pool(name="p_psr", bufs=2, space="PSUM"))

        feats = p_feats.tile([P, HKT, BC], dt_e)

        # constants (persist the whole kernel)
        ident_sb = p_const.tile([P, P], F32)
        nc.sync.dma_start(ident_sb[:], ident[:])
        ones1 = p_const.tile([1, P], F32)
        nc.vector.memset(ones1[:], 1.0)
        ones128 = p_const.tile([P, 1], F32)
        nc.vector.memset(ones128[:], 1.0)
        iota_sb = p_const.tile([1, E], F32)
        nc.sync.dma_start(iota_sb[:], iota8.rearrange("(a e) -> a e", a=1))
        eb3_sb = p_const.tile([E, CP], F32)
        nc.sync.dma_start(eb3_sb[:], eb3p[:])
        weightsT = p_const.tile([E, BC], F32)
        pk = p_const.tile([P, BC], F32)
        pkb = p_const.tile([P, 1], F32)
        fw2kw = p_const.tile([P, HKT, P], F32)

        def load_bias(bias_ap, mts):
            b = p_bias.tile([P, mts], F32, tag="bias")
            nc.sync.dma_start(b[:], bias_ap.rearrange("(mt p) -> p mt", p=P))
            return b

        def mm_chain(ps, wcol, kts, rhs_fn):
            for k in range(kts):
                nc.tensor.matmul(ps[:], wcol[:, k], rhs_fn(k),
                                 start=(k == 0), stop=(k == kts - 1))

        # ---- fold kw through fw2:  fw2kw = fw2 @ kw,  pkb = fb2 @ kw + kb ----
        # (pk is then computed from relu1 in full fp32, decoupling the routing
        #  chain's precision from the f32r L2.)
        with tc.tile_pool(name="p_fold", bufs=1) as p_fold:
            kwcol = p_fold.tile([P, HKT, P], F32, tag="kwcol")
            nc.sync.dma_start(kwcol[:], kwt[0])
            fb2_sb = load_bias(fb2, HMT)
            bk = load_bias(kb, 1)
            pkb_ps = p_psr.tile([P, 1], F32, tag="rps")
            for mt in range(HMT):
                nc.tensor.matmul(pkb_ps[:], kwcol[:, mt], fb2_sb[:, mt:mt + 1],
                                 start=(mt == 0), stop=(mt == HMT - 1))
            nc.vector.tensor_tensor(pkb[:], pkb_ps[:], bk[:, 0:1], ALU.add)
            for it in range(HMT):
                wcol = p_w.tile([P, HKT, P], F32, tag="wcol")
                nc.sync.dma_start(wcol[:], fw2Tt[it])
                ps = p_ps.tile([P, P], F32, tag="mmps")
                mm_chain(ps, wcol, HMT, lambda mt: kwcol[:, mt])
                nc.scalar.copy(fw2kw[:, it], ps[:])

        # ---- feature extractor (per batch half to bound SBUF) ----
        with tc.tile_pool(name="p_l12", bufs=1) as p_l12:
            x_sb = p_l12.tile([P, DKT, BC], F32, tag="x")
            nc.sync.dma_start(x_sb[:], xT.rearrange("(k p) b -> p k b", p=P))
            for nh2 in range(2):
                ns = slice(nh2 * NH, (nh2 + 1) * NH)
                r1h = p_l12.tile([P, HMT, NH], F32, tag="r1h")
                b1 = load_bias(fb1, HMT)
                for mt in range(HMT):
                    wcol = p_w.tile([P, DKT, P], F32, tag="wcol")
                    nc.sync.dma_start(wcol[:], fw1t[mt])
                    ps = p_ps.tile([P, NH], F32, tag="mmps")
                    mm_chain(ps, wcol, DKT, lambda k: x_sb[:, k, ns])
                    nc.scalar.activation(r1h[:, mt], ps[:], AF.Relu,
                                         bias=b1[:, mt:mt + 1])
                # routing projection from relu1, in fp32
                ps = p_ps.tile([P, NH], F32, tag="mmps")
                mm_chain(ps, fw2kw, HMT, lambda it: r1h[:, it])
                nc.scalar.activation(pk[:, ns], ps[:], AF.Identity,
                                     bias=pkb[:, 0:1])
                if dt_e != F32:
                    r1r = p_l12.tile([P, HMT, NH], dt_e, tag="r1r")
                    nc.scalar.activation(r1r[:], r1h[:], AF.Identity)
                else:
                    r1r = r1h
                b2 = load_bias(fb2, HMT)
                for mt in range(HMT):
                    wcol = p_w.tile([P, HKT, P], dt_e, tag="wcol")
                    nc.sync.dma_start(wcol[:], fw2t[mt])
                    ps = p_ps.tile([P, NH], F32, tag="mmps")
                    mm_chain(ps, wcol, HKT, lambda k: r1r[:, k])
                    nc.scalar.activation(feats[:, mt, ns], ps[:], AF.Identity,
                                         bias=b2[:, mt:mt + 1])

        # ---- routing ----
        with tc.tile_pool(name="p_route", bufs=2) as p_route:
            pksq = p_route.tile([P, BC], F32, tag="pksq")
            nc.vector.tensor_tensor(pksq[:], pk[:], pk[:], ALU.mult)

            keys_sb = p_route.tile([E, K], F32, tag="keys")
            nc.sync.dma_start(keys_sb[:], keys[:])
            ksq = p_route.tile([E, K], F32, tag="ksq")
            nc.vector.tensor_tensor(ksq[:], keys_sb[:], keys_sb[:], ALU.mult)
            kss = p_route.tile([E, 1], F32, tag="kss")
            nc.vector.reduce_sum(kss[:], ksq[:], axis=mybir.AxisListType.X)
            ksr = p_route.tile([E, 1], F32, tag="ksr")
            nc.scalar.activation(ksr[:], kss[:], AF.Sqrt)
            kin = p_route.tile([E, 1], F32, tag="kin")
            nc.vector.reciprocal(kin[:], ksr[:])
            keysn = p_route.tile([E, K], F32, tag="keysn")
            nc.vector.tensor_scalar_mul(keysn[:], keys_sb[:], kin[:])
            keysnT_ps = p_psr.tile([P, E], F32, tag="rps")
            nc.tensor.transpose(keysnT_ps[:], keysn[:], ident_sb[:E, :E])
            keysnT = p_route.tile([P, E], F32, tag="keysnT")
            nc.scalar.copy(keysnT[:], keysnT_ps[:])

            iota_ps = p_psr.tile([P, E], F32, tag="rps")
            nc.tensor.matmul(iota_ps[:], ones1[:], iota_sb[:], start=True,
                             stop=True)
            iota_bc = p_route.tile([P, E], F32, tag="iota_bc")
            nc.scalar.copy(iota_bc[:], iota_ps[:])

            for bt in range(BC // P):
                bs = slice(bt * P, (bt + 1) * P)
                ssq_ps = p_psr.tile([P, 1], F32, tag="rps")
                nc.tensor.matmul(ssq_ps[:], pksq[:, bs], ones128[:],
                                 start=True, stop=True)
                nrm = p_route.tile([P, 1], F32, tag="nrm")
                nc.scalar.activation(nrm[:], ssq_ps[:], AF.Sqrt)
                inv = p_route.tile([P, 1], F32, tag="inv")
                nc.vector.reciprocal(inv[:], nrm[:])

                sim_ps = p_psr.tile([P, E], F32, tag="rps")
                nc.tensor.matmul(sim_ps[:], pk[:, bs], keysnT[:], start=True,
                                 stop=True)
                sim_sb = p_route.tile([P, E], F32, tag="sim_sb")
                nc.vector.tensor_scalar_mul(sim_sb[:], sim_ps[:], inv[:])
                nc.sync.dma_start(simout[bs, :], sim_sb[:])

                vmax = p_route.tile([P, E], F32, tag="vmax")
                nc.vector.max(vmax[:], sim_sb[:])
                vidx = p_route.tile([P, E], U32, tag="vidx")
                nc.vector.max_index(vidx[:], vmax[:], sim_sb[:])

                negv1 = p_route.tile([P, 1], F32, tag="negv1")
                nc.vector.tensor_scalar_mul(negv1[:], vmax[:, 0:1], -1.0)
                e2 = p_route.tile([P, 1], F32, tag="e2")
                nc.scalar.activation(e2[:], vmax[:, 1:2], AF.Exp, bias=negv1[:])
                den = p_route.tile([P, 1], F32, tag="den")
                nc.vector.tensor_scalar_add(den[:], e2[:], 1.0)
                invd = p_route.tile([P, 1], F32, tag="invd")
                nc.vector.reciprocal(invd[:], den[:])
                sm2 = p_route.tile([P, 1], F32, tag="sm2")
                nc.vector.tensor_tensor(sm2[:], e2[:], invd[:], ALU.mult)

                idxf = p_route.tile([P, 2], F32, tag="idxf")
                nc.vector.tensor_copy(idxf[:], vidx[:, 0:2])
                iout_t = p_route.tile([P, 2], I32, tag="iout_t")
                nc.vector.tensor_copy(iout_t[:], vidx[:, 0:2])
                nc.sync.dma_start(iout[bs, :], iout_t[:])

                w1 = p_route.tile([P, E], F32, tag="w1")
                nc.vector.tensor_scalar(w1[:], iota_bc[:], idxf[:, 0:1],
                                        invd[:], ALU.is_equal, ALU.mult)
                w2 = p_route.tile([P, E], F32, tag="w2")
                nc.vector.tensor_scalar(w2[:], iota_bc[:], idxf[:, 1:2],
                                        sm2[:], ALU.is_equal, ALU.mult)
                w_sb = p_route.tile([P, E], F32, tag="w_sb")
                nc.vector.tensor_tensor(w_sb[:], w1[:], w2[:], ALU.add)
                nc.sync.dma_start(wout[bs, :], w_sb[:])

                wT_ps = p_psr.tile([E, P], F32, tag="rps")
                nc.tensor.transpose(wT_ps[:], w_sb[:], ident_sb[:])
                nc.scalar.copy(weightsT[:, bs], wT_ps[:])

        # ---- experts ----
        with tc.tile_pool(name="p_h1", bufs=1) as p_h1, \
             tc.tile_pool(name="p_h2w", bufs=1) as p_h2w, \
             tc.tile_pool(name="p_fin", bufs=1) as p_fin, \
             tc.tile_pool(name="p_wb", bufs=2) as p_wb:
            for nh in range(2):
                ns = slice(nh * NH, (nh + 1) * NH)
                fin = p_fin.tile([P, CMT, NH], F32, tag="fin")
                nc.vector.memset(fin[:], 0.0)

                for e in range(E):
                    eb1_sb = p_bias.tile([P, HMT], F32, tag="bias")
                    nc.sync.dma_start(
                        eb1_sb[:], eb1[e].rearrange("(mt p) -> p mt", p=P))
                    eb2_sb = p_bias.tile([P, HHMT], F32, tag="bias")
                    nc.sync.dma_start(
                        eb2_sb[:], eb2[e].rearrange("(mt p) -> p mt", p=P))

                    # w broadcast [P, NH] for this expert / batch half
                    # (stage row e at partition 0 — PE needs base partition 0)
                    wrow = p_wb.tile([1, NH], F32, tag="wrow")
                    nc.sync.dma_start(wrow[:], weightsT[e:e + 1, ns])
                    wb_ps = p_ps.tile([P, NH], F32, tag="mmps")
                    nc.tensor.matmul(wb_ps[:], ones1[:], wrow[:], start=True,
                                     stop=True)
                    wb = p_wb.tile([P, NH], dt_e, tag="wb")
                    nc.scalar.copy(wb[:], wb_ps[:])

                    h1 = p_h1.tile([P, HMT, NH], dt_e, tag="h1")
                    for mt in range(HMT):
                        wcol = p_w.tile([P, HKT, P], dt_e, tag="wcol")
                        nc.sync.dma_start(wcol[:], ew1t[e, mt])
                        ps = p_ps.tile([P, NH], F32, tag="mmps")
                        mm_chain(ps, wcol, HKT, lambda k: feats[:, k, ns])
                        nc.scalar.activation(h1[:, mt], ps[:], AF.Relu,
                                             bias=eb1_sb[:, mt:mt + 1])

                    h2w = p_h2w.tile([P, HHMT, NH], dt_e, tag="h2w")
                    for mt in range(HHMT):
                        wcol = p_w.tile([P, HKT, P], dt_e, tag="wcol")
                        nc.sync.dma_start(wcol[:], ew2t[e, mt])
                        ps = p_ps.tile([P, NH], F32, tag="mmps")
                        mm_chain(ps, wcol, HKT, lambda k: h1[:, k])
                        # relu(ps + eb2) then * w
                        nc.vector.tensor_scalar(h2w[:, mt], ps[:],
                                                eb2_sb[:, mt:mt + 1], 0.0,
                                                ALU.add, ALU.max)
                        nc.vector.tensor_tensor(h2w[:, mt], h2w[:, mt], wb[:],
                                                ALU.mult)

                    for mt in range(CMT):
                        wcol = p_w.tile([P, HHKT, P], dt_e, tag="wcol")
                        nc.sync.dma_start(wcol[:], ew3t[e, mt])
                        ps = p_ps.tile([P, NH], F32, tag="mmps")
                        mm_chain(ps, wcol, HHKT, lambda k: h2w[:, k])
                        nc.vector.tensor_tensor(fin[:, mt], fin[:, mt], ps[:],
                                                ALU.add)

                # bias-through-routing term and writeback
                for mt in range(CMT):
                    ps = p_ps.tile([P, NH], F32, tag="mmps")
                    nc.tensor.matmul(ps[:], eb3_sb[:, mt * P:(mt + 1) * P],
                                     weightsT[:, ns], start=True, stop=True)
                    nc.vector.tensor_tensor(fin[:, mt], fin[:, mt], ps[:],
                                            ALU.add)
                    nc.sync.dma_start(finalT[mt * P:(mt + 1) * P, ns],
                                      fin[:, mt])


def _retile(w, dt):
    """[K, M] f32 -> [M/P, P(of K), K/P, P(of M)] in dt."""
    Kd, Md = w.shape
    r = w.reshape(Kd // P, P, Md // P, P).transpose(2, 1, 0, 3)
    return np.ascontiguousarray(r).astype(dt, copy=False)


def _retile_e(w, dt):
    Ed, Kd, Md = w.shape
    r = w.reshape(Ed, Kd // P, P, Md // P, P).transpose(0, 3, 2, 1, 4)
    return np.ascontiguousarray(r).astype(dt, copy=False)


_CACHE = {}


def _get_nc(reps=1):
    key = ("nc", reps)
    if key not in _CACHE:
        _CACHE[key] = build_nc(reps)
    return _CACHE[key]


def _prep_shared(inputs):
    i = {k: np.asarray(v, dtype=np.float32) for k, v in inputs.items()}
    ew3 = np.zeros((E, HH, CP), np.float32)
    ew3[:, :, :C] = i["ew3"]
    eb3p = np.zeros((E, CP), np.float32)
    eb3p[:, :C] = i["eb3"]
    if DT_EXPERT == "bf16":
        import ml_dtypes
        edt = ml_dtypes.bfloat16
    else:
        edt = np.float32
    return {
        "fw1t": _retile(i["fw1"], np.float32),
        "fb1": i["fb1"],
        "fw2t": _retile(i["fw2"], edt if DT_EXPERT == "bf16" else np.float32),
        "fw2Tt": _retile(np.ascontiguousarray(i["fw2"].T), np.float32),
        "fb2": i["fb2"],
        "kwt": _retile(i["kw"], np.float32),
        "kb": i["kb"],
        "keys": i["keys"],
        "ew1t": _retile_e(i["ew1"], edt),
        "eb1": i["eb1"],
        "ew2t": _retile_e(i["ew2"], edt),
        "eb2": i["eb2"],
        "ew3t": _retile_e(ew3, edt),
        "eb3p": eb3p,
        "iota8": np.arange(E, dtype=np.float32),
        "ident": np.eye(P, dtype=np.float32),
    }


def _prep_inputs(inputs):
    shared = _prep_shared(inputs)
    xTf = np.ascontiguousarray(np.asarray(inputs["x"], np.float32).T)
    in_maps = []
    for c in range(NCORES):
        m = dict(shared)
        m["xT"] = np.ascontiguousarray(xTf[:, c * BC:(c + 1) * BC])
        in_maps.append(m)
    return in_maps


def _assemble(results):
    final = np.concatenate([r["finalT"][:C, :].T for r in results], axis=0)
    weights = np.concatenate([r["wout"] for r in results], axis=0)
    top_i = np.concatenate([r["iout"] for r in results], axis=0)
    sim = np.concatenate([r["simout"] for r in results], axis=0)
    return final, weights, top_i, sim


# ---------------------------------------------------------------------------
# Staged runner (mirrors concourse.bass2jax.run_bass_via_pjrt, but keeps the
# jitted executable and device-resident inputs so repeat calls only execute).
# ---------------------------------------------------------------------------

def _make_runner(reps=1):
    import jax
    from jax.experimental.shard_map import shard_map
    from jax.sharding import Mesh, PartitionSpec
    from concourse import bass2jax

    nc = _get_nc(reps)
    bass2jax.install_neuronx_cc_hook()
    pname = nc.partition_id_tensor.name if nc.partition_id_tensor else None
    in_names, out_names, out_avals, zero_outs = [], [], [], []
    for alloc in nc.m.functions[0].allocations:
        if not isinstance(alloc, mybir.MemoryLocationSet):
            continue
        name = alloc.memorylocations[0].name
        if alloc.kind == "ExternalInput":
            if name != pname:
                in_names.append(name)
        elif alloc.kind == "ExternalOutput":
            out_names.append(name)
            shape = tuple(alloc.tensor_shape)
            dtype = mybir.dt.np(alloc.dtype)
            out_avals.append(jax.core.ShapedArray(shape, dtype))
            zero_outs.append(np.zeros(shape, dtype))
    n_params = len(in_names)
    all_names = list(in_names) + list(out_names) + ([pname] if pname else [])

    def _body(*args):
        operands = list(args)
        if pname is not None:
            operands.append(bass2jax.partition_id_tensor())
        outs = bass2jax._bass_exec_p.bind(
            *operands,
            out_avals=tuple(out_avals),
            in_names=tuple(all_names),
            out_names=tuple(out_names),
            lowering_input_output_aliases=(),
            sim_require_finite=True,
            sim_require_nnan=True,
            nc=nc,
        )
        return tuple(outs)

    devices = jax.devices()[:NCORES]
    mesh = Mesh(np.asarray(devices), ("core",))
    in_specs = (PartitionSpec("core"),) * (n_params + len(out_names))
    out_specs = (PartitionSpec("core"),) * len(out_names)
    fn = jax.jit(
        shard_map(_body, mesh=mesh, in_specs=in_specs, out_specs=out_specs,
                  check_rep=False),
        keep_unused=True,
    )
    return dict(fn=fn, mesh=mesh, in_names=in_names, out_names=out_names,
                zero_outs=zero_outs, out_avals=out_avals)


def _get_runner(reps=1):
    key = ("runner", reps)
    if key not in _CACHE:
        _CACHE[key] = _make_runner(reps)
    return _CACHE[key]


def _stage(in_maps):
    import jax
    from jax.sharding import NamedSharding, PartitionSpec
    r = _get_runner()
    sh = NamedSharding(r["mesh"], PartitionSpec("core"))
    args = []
    for n in r["in_names"]:
        a = np.concatenate([np.asarray(m[n]) for m in in_maps], axis=0)
        args.append(jax.device_put(a, sh))
    for z in r["zero_outs"]:
        g = np.zeros((NCORES * z.shape[0], *z.shape[1:]), z.dtype)
        args.append(jax.device_put(g, sh))
    return args


def _run(args):
    r = _get_runner()
    outs = r["fn"](*args)
    res = []
    for c in range(NCORES):
        res.append({
            n: np.asarray(outs[i]).reshape(NCORES, *r["out_avals"][i].shape)[c]
            for i, n in enumerate(r["out_names"])
        })
    return outs, res


def kernel(**inputs):
    in_maps = _prep_inputs(inputs)
    args = _stage(in_maps)
    _, res = _run(args)
    return _assemble(res)


def time_kernel(inputs, iters=10):
    """Stages inputs on device, then times repeated executions (seconds)."""
    import jax
    in_maps = _prep_inputs(inputs)
    args = _stage(in_maps)
    r = _get_runner()
    outs = r["fn"](*args)          # warmup (compile happens on first call)
    jax.block_until_ready(outs)
    times = []
    for _ in range(iters):
        t0 = time.perf_counter()
        outs = r["fn"](*args)
        jax.block_until_ready(outs)
        times.append(time.perf_counter() - t0)
    return min(times), sorted(times)[len(times) // 2]


def measure_device_time(inputs, n_hi=9, reps=4):
    """Per-execution device time in seconds. Builds a second NEFF that runs
    the whole kernel body n_hi times back-to-back; the wall-clock difference
    vs the 1x NEFF cancels the ~100 ms axon dispatch floor exactly."""
    import jax
    in_maps = _prep_inputs(inputs)
    args = _stage(in_maps)

    def best(n):
        fn = _get_runner(n)["fn"]
        o = fn(*args)
        np.asarray(o[1])    # fetch forces completion (block_until_ready on
        ts = []             # axon returns before remote execution finishes)
        for _ in range(reps):
            t0 = time.perf_counter()
            o = fn(*args)
            np.asarray(o[1])
            ts.append(time.perf_counter() - t0)
        return min(ts)

    t_lo, t_hi = best(1), best(n_hi)
    return (t_hi - t_lo) / (n_hi - 1)


if __name__ == "__main__":
    t0 = time.time()
    nc = _get_nc()
    print("built in", round(time.time() - t0, 1), "s")
